# revision 15
# baseline (speedup 1.0000x reference)
"""CTC batch loss kernel for Trainium2 (8 NeuronCores, batch-parallel).

Math: reference computes logp = log_softmax(log(y+eps)) = log(y+eps) - log(rowsum),
then a log-space forward DP over the extended label sequence (S = 2L+1 = 129).
We run the DP in probability space with periodic renormalization, split into a
FORWARD chain (alpha, t=1..TSTAR) and a BACKWARD chain (beta, t=255..TSTAR+1)
that meet at TSTAR. Emission lanes are pre-divided by u_blank(t) (the blank
emission), which turns the blank-state updates into pure adds; the division
cancels in the final log-correction:
  loss[b] = sum_t [log rs(t) - log ub(t)] - sum_r log c_r - log(sum alpha~*beta~)

Per-core layout (32 samples/core):
  - y_pred transposed on host to [b, tq, c(part), cchunk, t]; per-sample gather
    matmul (contract over C) -> emis[t, lane], no on-chip transpose.
  - One-hot matrix O_b [1024, 130] per sample (host, bf16): lanes
    [ul(64) | su(64) | blank | ones]; su = skip-masked ul; ones lane = rowsum.
  - PE accumulates over 8 c-chunks in t-quarters of 64; ACT copies PSUM->SBUF
    bf16; SBUF->SBUF DMA repacks [64t,130] into emis[32b, 64t, 130]; GPSIMD
    scales lanes 0:128 by 1/ub(t) in bulk per quarter.
  - All DP ops are plain bf16 tensor_tensor add/mul (DVE 2x mode); fwd and bwd
    steps interleave [f1,b1,f2,b2,f3,b3,f4,b4] so each dependent pair is >= 2
    slots apart and the ~58-cycle SBUF latency is hidden.
      fwd: E(65)=alpha_even/prefix-ub, B=[q(64)|0|o(64)|0]:
        f1: q = o + E[0:64];  f2: E += [0|o];
        f3: XX = [q|0,o]*[ul'|su'];  f4: o = XX[0:64]+XX[64:128]
      bwd: BE(65), BO(64), G=[g_o(64)|0|h(64)|0]:
        b1: G = [BO*ul' | BO*su'] (broadcast-read BO)
        b2: T2 = BE[1:65]+G[66:130];  b3: BE += G[0:65];  b4: BO = T2+G[0:64]
  - Renorm by max every 32 steps per chain (+1 late fwd renorm pre-merge).
"""

import os
import sys
from contextlib import ExitStack

import numpy as np

sys.path.insert(0, "/opt/trn_rl_repo")
sys.path.insert(0, "/root/.axon_site/_ro/trn_rl_repo")

import ml_dtypes  # noqa: E402

B, T, C, L = 256, 256, 1024, 64
NCORES = 8
BS = B // NCORES  # 32 samples per core
NLANE = 130  # 64 ul | 64 su | blank | ones
KCH = C // 128  # 8 contraction chunks
NQ = 4
TQW = T // NQ  # 64
NORM_EVERY = 16
TSTAR = 142  # fwd computes alpha(1..TSTAR); bwd beta via t=255..TSTAR+1
FWD_RENORMS = list(range(15, 142, 16)) + [141]
NNF = len(FWD_RENORMS)  # 9
NNB = 7  # bwd renorms at backward-step index 15..111 step 16
BLANK = C - 1


# ---------------------------------------------------------------- host prep

def host_prep_y(y_pred: np.ndarray) -> np.ndarray:
    """[B, T, C] f32 -> [B, NQ, 128(c part), KCH, TQW(t)] f32 contiguous."""
    yt = y_pred.reshape(B, NQ, TQW, KCH, 128).transpose(0, 1, 4, 3, 2)
    return np.ascontiguousarray(yt)


def host_prep_oh(y_true: np.ndarray) -> np.ndarray:
    """[B, L] int -> one-hot+aux matrix [B, 128(c part), KCH, NLANE] bf16."""
    lab = y_true.astype(np.int64)
    oh = np.zeros((B, C, NLANE), dtype=np.float32)
    bidx = np.arange(B)[:, None]
    jidx = np.arange(L)[None, :]
    oh[bidx, lab, jidx] = 1.0  # ul lanes
    skip = np.zeros((B, L), dtype=np.float32)
    skip[:, 1:] = (lab[:, 1:] != lab[:, :-1]).astype(np.float32)
    oh[bidx, lab, jidx + L] = skip  # su lanes
    oh[:, BLANK, 2 * L] = 1.0  # blank lane
    oh[:, :, 2 * L + 1] = 1.0  # ones lane (rowsum)
    oh = oh.reshape(B, KCH, 128, NLANE).transpose(0, 2, 1, 3)
    return np.ascontiguousarray(oh).astype(ml_dtypes.float8_e4m3)


# ---------------------------------------------------------------- bass build

def build_nc():
    import concourse.bass as bass
    import concourse.tile as tile
    from concourse import bacc, mybir

    f32 = mybir.dt.float32
    bf16 = mybir.dt.bfloat16
    f8 = mybir.dt.float8e4

    nc = bacc.Bacc(None, target_bir_lowering=False)

    yt_d = nc.declare_dram_parameter("yt", [BS, NQ, 128, KCH, TQW], f32, isOutput=False)
    oh_d = nc.declare_dram_parameter("oh", [BS, 128, KCH, NLANE], f8, isOutput=False)
    out_d = nc.declare_dram_parameter("out", [BS, 1], f32, isOutput=True)

    with tile.TileContext(nc) as tc:
        with ExitStack() as ctx:
            ohp = ctx.enter_context(tc.tile_pool(name="ohp", bufs=1))
            yp = ctx.enter_context(tc.tile_pool(name="yp", bufs=10))
            psp = ctx.enter_context(
                tc.tile_pool(name="psp", bufs=4, space=bass.MemorySpace.PSUM)
            )
            stp = ctx.enter_context(tc.tile_pool(name="stp", bufs=4))
            emp = ctx.enter_context(tc.tile_pool(name="emp", bufs=1))
            alp = ctx.enter_context(tc.tile_pool(name="alp", bufs=1))
            fin = ctx.enter_context(tc.tile_pool(name="fin", bufs=1))

            oh_sb = [None] * BS

            # persistent DP state (single buffered; updates are in-place safe)
            EF = alp.tile([BS, L + 1], bf16, name="ef")  # fwd even-tilde (65)
            BF = alp.tile([BS, 2 * L + 2], bf16, name="bf")  # [q|0|o|0] (130)
            XX = alp.tile([BS, 2 * L], bf16, name="xx")
            BE = alp.tile([BS, L + 1], bf16, name="be")  # beta even (65)
            BO = alp.tile([BS, L], bf16, name="bo")  # beta odd (64)
            G = alp.tile([BS, 2 * L + 2], bf16, name="g")  # [g_o|0|h|0] (130)
            T2 = alp.tile([BS, L], bf16, name="t2")
            UBT = fin.tile([BS, T], f32)  # raw ub per t
            RST = fin.tile([BS, T], f32)  # raw rowsum per t
            RCB = fin.tile([BS, T], f32)  # 1/ub
            NRM = fin.tile([BS, NNF + NNB], f32)
            TMPM = alp.tile([BS, 1], f32, name="tmpm")
            TMPR = alp.tile([BS, 1], f32, name="tmpr")

            for t_ in (EF, BF, XX, BE, BO, G, T2):
                nc.vector.memset(t_[:], 0.0)
            nc.vector.memset(EF[:, 0:1], 1.0)  # e~(0) = [1,0..]
            nc.vector.memset(BE[:, L : L + 1], 1.0)  # beta_e[64] = 1 (s=128)
            nc.vector.memset(BO[:, L - 1 : L], 1.0)  # beta_o[63] = 1 (s=127)

            em_sb = {}

            def produce(q, load_oh=False):
                em = emp.tile([BS, TQW, NLANE], bf16, tag=f"em{q}", name=f"em{q}")
                em_sb[q] = em
                for b in range(BS):
                    if load_oh:
                        t_oh = ohp.tile(
                            [128, KCH, NLANE], f8, tag=f"oh{b}", name=f"oh{b}"
                        )
                        nc.sync.dma_start(t_oh[:], oh_d[b])
                        oh_sb[b] = t_oh
                    ybf = yp.tile([128, KCH, TQW], bf16, tag="ybf", name="ybf")
                    nc.gpsimd.dma_start(ybf[:], yt_d[b, q])  # f32->bf16 cast DMA
                    ps = psp.tile([TQW, NLANE], f32, tag="ps", name="ps")
                    for k in range(KCH):
                        nc.tensor.matmul(
                            ps[:], ybf[:, k, :], oh_sb[b][:, k, :],
                            start=(k == 0), stop=(k == KCH - 1),
                        )
                    st = stp.tile([TQW, NLANE], bf16, tag="st", name="st")
                    nc.scalar.copy(st[:], ps[:])
                    nc.sync.dma_start(em[b : b + 1], st[:])

            def prep(q):
                """Extract raw ub/rs lanes; compute 1/ub (contiguous)."""
                qr = slice(q * TQW, (q + 1) * TQW)
                em = em_sb[q]
                nc.vector.tensor_single_scalar(
                    UBT[:, qr], em[:, :, 2 * L], 1e-30, mybir.AluOpType.max
                )
                nc.vector.tensor_copy(RST[:, qr], em[:, :, 2 * L + 1])
                nc.vector.reciprocal(RCB[:, qr], UBT[:, qr])
                nc.gpsimd.tensor_mul(
                    em[:, :, 0 : 2 * L],
                    em[:, :, 0 : 2 * L],
                    RCB[:, qr, None].broadcast_to([BS, TQW, 2 * L]),
                )

            def renorm(a65, b64, r):
                nc.vector.tensor_reduce(
                    TMPM[:], a65[:], mybir.AxisListType.X, mybir.AluOpType.max
                )
                nc.vector.tensor_reduce(
                    NRM[:, r : r + 1], b64[:], mybir.AxisListType.X,
                    mybir.AluOpType.max,
                )
                nc.vector.tensor_max(NRM[:, r : r + 1], NRM[:, r : r + 1], TMPM[:])
                nc.vector.reciprocal(TMPR[:], NRM[:, r : r + 1])
                nc.vector.tensor_scalar_mul(a65[:], a65[:], TMPR[:])
                nc.vector.tensor_scalar_mul(b64[:], b64[:], TMPR[:])

            def femit(t, phase):
                """Forward step t, op index phase (0..3)."""
                em = em_sb[t // TQW]
                tt = t % TQW
                if phase == 0:  # q = o + E[0:64]
                    nc.vector.tensor_add(
                        BF[:, 0:L], BF[:, L + 1 : 2 * L + 1], EF[:, 0:L]
                    )
                elif phase == 1:  # E += [0|o]  (in place; before f4 rewrites o)
                    nc.vector.tensor_add(EF[:], EF[:], BF[:, L : 2 * L + 1])
                elif phase == 2:  # XX = [q|0,o] * [ul'|su']
                    nc.vector.tensor_mul(
                        XX[:], BF[:, 0 : 2 * L], em[:, tt, 0 : 2 * L]
                    )
                else:  # o = XX[0:64] + XX[64:128]
                    nc.vector.tensor_add(
                        BF[:, L + 1 : 2 * L + 1], XX[:, 0:L], XX[:, L : 2 * L]
                    )
                    if t in FWD_RENORMS:
                        renorm(EF, BF[:, L + 1 : 2 * L + 1], FWD_RENORMS.index(t))

            def bemit(t, phase):
                """Backward step consuming emissions at t, op index phase."""
                em = em_sb[t // TQW]
                tt = t % TQW
                if phase == 0:  # G = [(BO/ub)*ul | (BO/ub)*su]
                    g2 = G[:, 0 : 2 * (L + 1)].rearrange(
                        "p (a b) -> p a b", a=2, b=L + 1
                    )[:, :, 0:L]
                    bo2 = BO[:, None, 0:L].broadcast_to([BS, 2, L])
                    em2 = em[:, tt, 0 : 2 * L].rearrange("p (a b) -> p a b", a=2, b=L)
                    nc.vector.tensor_mul(g2, bo2, em2)
                elif phase == 1:  # T2 = BE[1:65] + h[j+1]
                    nc.vector.tensor_add(
                        T2[:], BE[:, 1 : L + 1], G[:, L + 2 : 2 * L + 2]
                    )
                elif phase == 2:  # BE += [g_o|0]  (in place)
                    nc.vector.tensor_add(BE[:], BE[:], G[:, 0 : L + 1])
                else:  # BO = T2 + g_o
                    nc.vector.tensor_add(BO[:], T2[:], G[:, 0:L])
                    bi = 255 - t
                    if bi % NORM_EVERY == NORM_EVERY - 1:
                        renorm(BE, BO, NNF + bi // NORM_EVERY)

            def fwd_step(t):
                for ph in range(4):
                    femit(t, ph)

            # ---- emission schedule ----
            produce(0, load_oh=True)
            prep(0)
            # init: o~(0)[0] = ul'(0)[0] (em lane already scaled by 1/ub)
            nc.vector.tensor_copy(BF[:, L + 1 : L + 2], em_sb[0][:, 0, 0:1])
            for t in range(1, 31):
                fwd_step(t)
            produce(1)
            prep(1)
            produce(3)
            prep(3)
            produce(2)
            fwd_list = list(range(31, TSTAR + 1))  # 112 steps
            bwd_list = list(range(255, TSTAR, -1))  # 113 steps
            np_pairs = max(len(fwd_list), len(bwd_list))
            for i in range(np_pairs):
                if i == 50:
                    prep(2)
                ft = fwd_list[i] if i < len(fwd_list) else None
                bt = bwd_list[i] if i < len(bwd_list) else None
                for ph in range(4):
                    if ft is not None:
                        femit(ft, ph)
                    if bt is not None:
                        bemit(bt, ph)

            # ---- merge at TSTAR: L~ = sum(E*BE) + sum(o*BO)
            M1 = fin.tile([BS, L + 1], f32)
            M2 = fin.tile([BS, L], f32)
            R1 = fin.tile([BS, 1], f32)
            LS = fin.tile([BS, 1], f32)
            nc.vector.tensor_mul(M1[:], EF[:], BE[:])
            nc.vector.tensor_mul(M2[:], BF[:, L + 1 : 2 * L + 1], BO[:])
            nc.vector.tensor_reduce(
                R1[:], M1[:], mybir.AxisListType.X, mybir.AluOpType.add
            )
            nc.vector.tensor_reduce(
                LS[:], M2[:], mybir.AxisListType.X, mybir.AluOpType.add
            )
            nc.vector.tensor_add(LS[:], LS[:], R1[:])
            ln_ls = fin.tile([BS, 1], f32)
            nc.scalar.activation(ln_ls[:], LS[:], mybir.ActivationFunctionType.Ln)
            scr_n = fin.tile([BS, NNF + NNB], f32)
            acc_n = fin.tile([BS, 1], f32)
            nc.scalar.activation(
                scr_n[:], NRM[:], mybir.ActivationFunctionType.Ln,
                scale=float(2.0 ** -16), accum_out=acc_n[:]
            )
            scr_r = fin.tile([BS, T], f32)
            acc_r = fin.tile([BS, 1], f32)
            nc.scalar.activation(
                scr_r[:], RST[:], mybir.ActivationFunctionType.Ln,
                accum_out=acc_r[:],
            )
            scr_u = fin.tile([BS, T], f32)
            acc_u = fin.tile([BS, 1], f32)
            nc.scalar.activation(
                scr_u[:], UBT[:], mybir.ActivationFunctionType.Ln,
                accum_out=acc_u[:],
            )
            # loss = (acc_r - acc_u) - acc_n - ln_ls
            loss = fin.tile([BS, 1], f32)
            nc.vector.tensor_sub(loss[:], acc_r[:], acc_u[:])
            nc.vector.tensor_sub(loss[:], loss[:], acc_n[:])
            nc.vector.tensor_sub(loss[:], loss[:], ln_ls[:])
            # acc_n used Ln(m * 2^-16); add back (NNF+NNB)*16*ln2
            import math
            nc.vector.tensor_single_scalar(
                loss[:], loss[:], float((NNF + NNB) * 16.0 * math.log(2.0)),
                mybir.AluOpType.subtract,
            )
            nc.sync.dma_start(out_d[:], loss[:])

    nc._dbg = {
        "EF": EF.name, "BF": BF.name, "XX": XX.name, "BE": BE.name,
        "BO": BO.name, "G": G.name, "T2": T2.name, "UBT": UBT.name,
        "RST": RST.name, "RCB": RCB.name, "NRM": NRM.name,
        "em": {q: em_sb[q].name for q in em_sb},
    }
    nc.compile()
    return nc


_NC_CACHE = {}


def _get_nc():
    if "nc" not in _NC_CACHE:
        _NC_CACHE["nc"] = build_nc()
    return _NC_CACHE["nc"]


# ---------------------------------------------------------------- entrypoint

def kernel(y_true: np.ndarray, y_pred: np.ndarray, _trace: bool = False):
    from concourse.bass_utils import run_bass_kernel_spmd

    yt = host_prep_y(np.asarray(y_pred, dtype=np.float32))
    oh = host_prep_oh(np.asarray(y_true))

    in_maps = []
    for i in range(NCORES):
        sl = slice(i * BS, (i + 1) * BS)
        in_maps.append({"yt": yt[sl], "oh": oh[sl]})

    nc = _get_nc()
    res = run_bass_kernel_spmd(nc, in_maps, list(range(NCORES)), trace=_trace)
    out = np.concatenate([res.results[i]["out"] for i in range(NCORES)], axis=0)
    if _trace:
        return out.astype(np.float32), res
    return out.astype(np.float32)


# revision 16
# speedup vs baseline: 1.0931x; 1.0931x over previous
"""CTC batch loss kernel for Trainium2 (8 NeuronCores, batch-parallel).

Math: reference computes logp = log_softmax(log(y+eps)) = log(y+eps) - log(rowsum),
then a log-space forward DP over the extended label sequence (S = 2L+1 = 129).
We run the DP in probability space with periodic renormalization, split into a
FORWARD chain (alpha, t=1..TSTAR) and a BACKWARD chain (beta, t=255..TSTAR+1)
that meet at TSTAR. Emission lanes are pre-divided by u_blank(t) (the blank
emission), which turns the blank-state updates into pure adds; the division
cancels in the final log-correction:
  loss[b] = sum_t [log rs(t) - log ub(t)] - sum_r log c_r - log(sum alpha~*beta~)

Per-core layout (32 samples/core):
  - y_pred transposed on host to [b, tq, c(part), cchunk, t]; per-sample gather
    matmul (contract over C) -> emis[t, lane], no on-chip transpose.
  - One-hot matrix O_b [1024, 130] per sample (host, bf16): lanes
    [ul(64) | su(64) | blank | ones]; su = skip-masked ul; ones lane = rowsum.
  - PE accumulates over 8 c-chunks in t-quarters of 64; ACT copies PSUM->SBUF
    bf16; SBUF->SBUF DMA repacks [64t,130] into emis[32b, 64t, 130]; GPSIMD
    scales lanes 0:128 by 1/ub(t) in bulk per quarter.
  - All DP ops are plain bf16 tensor_tensor add/mul (DVE 2x mode); fwd and bwd
    steps interleave [f1,b1,f2,b2,f3,b3,f4,b4] so each dependent pair is >= 2
    slots apart and the ~58-cycle SBUF latency is hidden.
      fwd: E(65)=alpha_even/prefix-ub, B=[q(64)|0|o(64)|0]:
        f1: q = o + E[0:64];  f2: E += [0|o];
        f3: XX = [q|0,o]*[ul'|su'];  f4: o = XX[0:64]+XX[64:128]
      bwd: BE(65), BO(64), G=[g_o(64)|0|h(64)|0]:
        b1: G = [BO*ul' | BO*su'] (broadcast-read BO)
        b2: T2 = BE[1:65]+G[66:130];  b3: BE += G[0:65];  b4: BO = T2+G[0:64]
  - Renorm by max every 32 steps per chain (+1 late fwd renorm pre-merge).
"""

import os
import sys
from contextlib import ExitStack

import numpy as np

sys.path.insert(0, "/opt/trn_rl_repo")
sys.path.insert(0, "/root/.axon_site/_ro/trn_rl_repo")

import ml_dtypes  # noqa: E402

B, T, C, L = 256, 256, 1024, 64
NCORES = 8
BS = B // NCORES  # 32 samples per core
NLANE = 130  # 64 ul | 64 su | blank | ones
KCH = C // 128  # 8 contraction chunks
NQ = 4
TQW = T // NQ  # 64
NORM_EVERY = 16
TSTAR = 142  # fwd computes alpha(1..TSTAR); bwd beta via t=255..TSTAR+1
FWD_RENORMS = list(range(15, 142, 16)) + [141]
NNF = len(FWD_RENORMS)  # 9
NNB = 7  # bwd renorms at backward-step index 15..111 step 16
BLANK = C - 1


# ---------------------------------------------------------------- host prep

def host_prep_y(y_pred: np.ndarray) -> np.ndarray:
    """[B, T, C] f32 -> [B, NQ, 128(c part), KCH, TQW(t)] f32 contiguous."""
    yt = y_pred.reshape(B, NQ, TQW, KCH, 128).transpose(0, 1, 4, 3, 2)
    return np.ascontiguousarray(yt)


def host_prep_oh(y_true: np.ndarray) -> np.ndarray:
    """[B, L] int -> one-hot+aux matrix [B, 128(c part), KCH, NLANE] bf16."""
    lab = y_true.astype(np.int64)
    oh = np.zeros((B, C, NLANE), dtype=np.float32)
    bidx = np.arange(B)[:, None]
    jidx = np.arange(L)[None, :]
    oh[bidx, lab, jidx] = 1.0  # ul lanes
    skip = np.zeros((B, L), dtype=np.float32)
    skip[:, 1:] = (lab[:, 1:] != lab[:, :-1]).astype(np.float32)
    oh[bidx, lab, jidx + L] = skip  # su lanes
    oh[:, BLANK, 2 * L] = 1.0  # blank lane
    oh[:, :, 2 * L + 1] = 1.0  # ones lane (rowsum)
    oh = oh.reshape(B, KCH, 128, NLANE).transpose(0, 2, 1, 3)
    return np.ascontiguousarray(oh).astype(ml_dtypes.bfloat16)


# ---------------------------------------------------------------- bass build

def build_nc():
    import concourse.bass as bass
    import concourse.tile as tile
    from concourse import bacc, mybir

    f32 = mybir.dt.float32
    bf16 = mybir.dt.bfloat16
    f8 = mybir.dt.float8e4

    nc = bacc.Bacc(None, target_bir_lowering=False)

    yt_d = nc.declare_dram_parameter("yt", [BS, NQ, 128, KCH, TQW], f32, isOutput=False)
    oh_d = nc.declare_dram_parameter("oh", [BS, 128, KCH, NLANE], bf16, isOutput=False)
    out_d = nc.declare_dram_parameter("out", [BS, 1], f32, isOutput=True)

    with tile.TileContext(nc) as tc:
        with ExitStack() as ctx:
            ohp = ctx.enter_context(tc.tile_pool(name="ohp", bufs=1))
            yp = ctx.enter_context(tc.tile_pool(name="yp", bufs=10))
            psp = ctx.enter_context(
                tc.tile_pool(name="psp", bufs=4, space=bass.MemorySpace.PSUM)
            )
            stp = ctx.enter_context(tc.tile_pool(name="stp", bufs=4))
            emp = ctx.enter_context(tc.tile_pool(name="emp", bufs=1))
            alp = ctx.enter_context(tc.tile_pool(name="alp", bufs=1))
            fin = ctx.enter_context(tc.tile_pool(name="fin", bufs=1))

            oh_sb = [None] * BS

            # persistent DP state (single buffered; updates are in-place safe)
            EF = alp.tile([BS, L + 1], bf16, name="ef")  # fwd even-tilde (65)
            BF = alp.tile([BS, 2 * L + 2], bf16, name="bf")  # [q|0|o|0] (130)
            XX = alp.tile([BS, 2 * L], bf16, name="xx")
            BE = alp.tile([BS, L + 1], bf16, name="be")  # beta even (65)
            BO = alp.tile([BS, L], bf16, name="bo")  # beta odd (64)
            G = alp.tile([BS, 2 * L + 2], bf16, name="g")  # [g_o|0|h|0] (130)
            T2 = alp.tile([BS, L], bf16, name="t2")
            UBT = fin.tile([BS, T], f32)  # raw ub per t
            RST = fin.tile([BS, T], f32)  # raw rowsum per t
            RCB = fin.tile([BS, T], f32)  # 1/ub
            NRM = fin.tile([BS, NNF + NNB], f32)
            TMPM = alp.tile([BS, 1], f32, name="tmpm")
            TMPR = alp.tile([BS, 1], f32, name="tmpr")

            for t_ in (EF, BF, XX, BE, BO, G, T2):
                nc.vector.memset(t_[:], 0.0)
            nc.vector.memset(EF[:, 0:1], 1.0)  # e~(0) = [1,0..]
            nc.vector.memset(BE[:, L : L + 1], 1.0)  # beta_e[64] = 1 (s=128)
            nc.vector.memset(BO[:, L - 1 : L], 1.0)  # beta_o[63] = 1 (s=127)

            em_sb = {}

            def produce(q, load_oh=False):
                em = emp.tile([BS, TQW, NLANE], bf16, tag=f"em{q}", name=f"em{q}")
                em_sb[q] = em
                for b in range(BS):
                    if load_oh:
                        t_oh = ohp.tile(
                            [128, KCH, NLANE], bf16, tag=f"oh{b}", name=f"oh{b}"
                        )
                        nc.sync.dma_start(t_oh[:], oh_d[b])
                        oh_sb[b] = t_oh
                    ybf = yp.tile([128, KCH, TQW], bf16, tag="ybf", name="ybf")
                    nc.gpsimd.dma_start(ybf[:], yt_d[b, q])  # f32->bf16 cast DMA
                    ps = psp.tile([TQW, NLANE], f32, tag="ps", name="ps")
                    for k in range(KCH):
                        nc.tensor.matmul(
                            ps[:], ybf[:, k, :], oh_sb[b][:, k, :],
                            start=(k == 0), stop=(k == KCH - 1),
                        )
                    st = stp.tile([TQW, NLANE], bf16, tag="st", name="st")
                    nc.scalar.copy(st[:], ps[:])
                    nc.sync.dma_start(em[b : b + 1], st[:])

            def prep(q):
                """Extract raw ub/rs lanes; compute 1/ub (contiguous)."""
                qr = slice(q * TQW, (q + 1) * TQW)
                em = em_sb[q]
                nc.vector.tensor_single_scalar(
                    UBT[:, qr], em[:, :, 2 * L], 1e-30, mybir.AluOpType.max
                )
                nc.vector.tensor_copy(RST[:, qr], em[:, :, 2 * L + 1])
                nc.vector.reciprocal(RCB[:, qr], UBT[:, qr])
                sub = TQW // 4
                for j in range(4):
                    ts0 = j * sub
                    nc.vector.tensor_mul(
                        em[:, ts0 : ts0 + sub, 0 : 2 * L],
                        em[:, ts0 : ts0 + sub, 0 : 2 * L],
                        RCB[:, q * TQW + ts0 : q * TQW + ts0 + sub, None]
                        .broadcast_to([BS, sub, 2 * L]),
                    )

            def renorm(a65, b64, r):
                nc.vector.tensor_reduce(
                    TMPM[:], a65[:], mybir.AxisListType.X, mybir.AluOpType.max
                )
                nc.vector.tensor_reduce(
                    NRM[:, r : r + 1], b64[:], mybir.AxisListType.X,
                    mybir.AluOpType.max,
                )
                nc.vector.tensor_max(NRM[:, r : r + 1], NRM[:, r : r + 1], TMPM[:])
                nc.vector.reciprocal(TMPR[:], NRM[:, r : r + 1])
                nc.vector.tensor_scalar_mul(a65[:], a65[:], TMPR[:])
                nc.vector.tensor_scalar_mul(b64[:], b64[:], TMPR[:])

            def femit(t, phase):
                """Forward step t, op index phase (0..3)."""
                em = em_sb[t // TQW]
                tt = t % TQW
                if phase == 0:  # q = o + E[0:64]
                    nc.vector.tensor_add(
                        BF[:, 0:L], BF[:, L + 1 : 2 * L + 1], EF[:, 0:L]
                    )
                elif phase == 1:  # E += [0|o]  (in place; before f4 rewrites o)
                    nc.vector.tensor_add(EF[:], EF[:], BF[:, L : 2 * L + 1])
                elif phase == 2:  # XX = [q|0,o] * [ul'|su']
                    nc.vector.tensor_mul(
                        XX[:], BF[:, 0 : 2 * L], em[:, tt, 0 : 2 * L]
                    )
                else:  # o = XX[0:64] + XX[64:128]
                    nc.vector.tensor_add(
                        BF[:, L + 1 : 2 * L + 1], XX[:, 0:L], XX[:, L : 2 * L]
                    )
                    if t in FWD_RENORMS:
                        renorm(EF, BF[:, L + 1 : 2 * L + 1], FWD_RENORMS.index(t))

            def bemit(t, phase):
                """Backward step consuming emissions at t, op index phase."""
                em = em_sb[t // TQW]
                tt = t % TQW
                if phase == 0:  # G = [(BO/ub)*ul | (BO/ub)*su]
                    g2 = G[:, 0 : 2 * (L + 1)].rearrange(
                        "p (a b) -> p a b", a=2, b=L + 1
                    )[:, :, 0:L]
                    bo2 = BO[:, None, 0:L].broadcast_to([BS, 2, L])
                    em2 = em[:, tt, 0 : 2 * L].rearrange("p (a b) -> p a b", a=2, b=L)
                    nc.vector.tensor_mul(g2, bo2, em2)
                elif phase == 1:  # T2 = BE[1:65] + h[j+1]
                    nc.vector.tensor_add(
                        T2[:], BE[:, 1 : L + 1], G[:, L + 2 : 2 * L + 2]
                    )
                elif phase == 2:  # BE += [g_o|0]  (in place)
                    nc.vector.tensor_add(BE[:], BE[:], G[:, 0 : L + 1])
                else:  # BO = T2 + g_o
                    nc.vector.tensor_add(BO[:], T2[:], G[:, 0:L])
                    bi = 255 - t
                    if bi % NORM_EVERY == NORM_EVERY - 1:
                        renorm(BE, BO, NNF + bi // NORM_EVERY)

            def fwd_step(t):
                for ph in range(4):
                    femit(t, ph)

            # ---- emission schedule ----
            produce(0, load_oh=True)
            prep(0)
            # init: o~(0)[0] = ul'(0)[0] (em lane already scaled by 1/ub)
            nc.vector.tensor_copy(BF[:, L + 1 : L + 2], em_sb[0][:, 0, 0:1])
            for t in range(1, 31):
                fwd_step(t)
            produce(1)
            prep(1)
            produce(3)
            prep(3)
            produce(2)
            fwd_list = list(range(31, TSTAR + 1))  # 112 steps
            bwd_list = list(range(255, TSTAR, -1))  # 113 steps
            np_pairs = max(len(fwd_list), len(bwd_list))
            for i in range(np_pairs):
                if i == 50:
                    prep(2)
                ft = fwd_list[i] if i < len(fwd_list) else None
                bt = bwd_list[i] if i < len(bwd_list) else None
                for ph in range(4):
                    if ft is not None:
                        femit(ft, ph)
                    if bt is not None:
                        bemit(bt, ph)

            # ---- merge at TSTAR: L~ = sum(E*BE) + sum(o*BO)
            M1 = fin.tile([BS, L + 1], f32)
            M2 = fin.tile([BS, L], f32)
            R1 = fin.tile([BS, 1], f32)
            LS = fin.tile([BS, 1], f32)
            nc.vector.tensor_mul(M1[:], EF[:], BE[:])
            nc.vector.tensor_mul(M2[:], BF[:, L + 1 : 2 * L + 1], BO[:])
            nc.vector.tensor_reduce(
                R1[:], M1[:], mybir.AxisListType.X, mybir.AluOpType.add
            )
            nc.vector.tensor_reduce(
                LS[:], M2[:], mybir.AxisListType.X, mybir.AluOpType.add
            )
            nc.vector.tensor_add(LS[:], LS[:], R1[:])
            ln_ls = fin.tile([BS, 1], f32)
            nc.scalar.activation(ln_ls[:], LS[:], mybir.ActivationFunctionType.Ln)
            scr_n = fin.tile([BS, NNF + NNB], f32)
            acc_n = fin.tile([BS, 1], f32)
            nc.scalar.activation(
                scr_n[:], NRM[:], mybir.ActivationFunctionType.Ln,
                scale=float(2.0 ** -16), accum_out=acc_n[:]
            )
            scr_r = fin.tile([BS, T], f32)
            acc_r = fin.tile([BS, 1], f32)
            nc.scalar.activation(
                scr_r[:], RST[:], mybir.ActivationFunctionType.Ln,
                accum_out=acc_r[:],
            )
            scr_u = fin.tile([BS, T], f32)
            acc_u = fin.tile([BS, 1], f32)
            nc.scalar.activation(
                scr_u[:], UBT[:], mybir.ActivationFunctionType.Ln,
                accum_out=acc_u[:],
            )
            # loss = (acc_r - acc_u) - acc_n - ln_ls
            loss = fin.tile([BS, 1], f32)
            nc.vector.tensor_sub(loss[:], acc_r[:], acc_u[:])
            nc.vector.tensor_sub(loss[:], loss[:], acc_n[:])
            nc.vector.tensor_sub(loss[:], loss[:], ln_ls[:])
            # acc_n used Ln(m * 2^-16); add back (NNF+NNB)*16*ln2
            import math
            nc.vector.tensor_single_scalar(
                loss[:], loss[:], float((NNF + NNB) * 16.0 * math.log(2.0)),
                mybir.AluOpType.subtract,
            )
            nc.sync.dma_start(out_d[:], loss[:])

    nc._dbg = {
        "EF": EF.name, "BF": BF.name, "XX": XX.name, "BE": BE.name,
        "BO": BO.name, "G": G.name, "T2": T2.name, "UBT": UBT.name,
        "RST": RST.name, "RCB": RCB.name, "NRM": NRM.name,
        "em": {q: em_sb[q].name for q in em_sb},
    }
    nc.compile()
    return nc


_NC_CACHE = {}


def _get_nc():
    if "nc" not in _NC_CACHE:
        _NC_CACHE["nc"] = build_nc()
    return _NC_CACHE["nc"]


# ---------------------------------------------------------------- entrypoint

def kernel(y_true: np.ndarray, y_pred: np.ndarray, _trace: bool = False):
    from concourse.bass_utils import run_bass_kernel_spmd

    yt = host_prep_y(np.asarray(y_pred, dtype=np.float32))
    oh = host_prep_oh(np.asarray(y_true))

    in_maps = []
    for i in range(NCORES):
        sl = slice(i * BS, (i + 1) * BS)
        in_maps.append({"yt": yt[sl], "oh": oh[sl]})

    nc = _get_nc()
    res = run_bass_kernel_spmd(nc, in_maps, list(range(NCORES)), trace=_trace)
    out = np.concatenate([res.results[i]["out"] for i in range(NCORES)], axis=0)
    if _trace:
        return out.astype(np.float32), res
    return out.astype(np.float32)
